# revision 9
# baseline (speedup 1.0000x reference)
"""Trainium2 Bass kernel for nn_ClusterOverlap (retrieval_knn) — v4 final.

Math identical to v2 (fp32r GEMM with a K=1 fold of -e2, fp16 top-26
threshold via max8+match_replace, strict mask, PE-transposed matmul
histogram, entropy x confidence); restructured for engine occupancy.
203071ns -> 158167ns (CoreSim cost model; rel err 2.7e-3 vs 2e-2 gate).

  - e2 path with no DRAM roundtrip: Pool squares+sums the transposed E
    chunks, a (-1)-column matmul reduces partitions to -e2 [1,512], ACT
    evacuates Copy bias=+256 into an fp16 e2row = (256 - e2).  The fold
    adds it in-PSUM (fp16 lhsT ones, 16-bit pair — walrus rejects mixed
    16/32-bit matmul inputs), and every GEMM evacuation applies the
    remaining -80 of the fp16 recenter as a free bias, so evacuations
    stay engine-flexible (ACT Copy / DVE tensor_scalar).
  - E prep in 16 x 512-column chunks, pipelined (psum pt pool bufs=3);
    blocks 0/1's GEMM interleaves per chunk pair to keep PE fed.
  - onehot[j,c] = (cat[j,c] == rowmax[j]) value-match on Pool — no DVE
    argmax chain (ties are measure-zero for the uniform input).
  - per-block indirect row gathers ([128,1] offsets only: wider offset
    APs NaN through walrus even though CoreSim accepts them).
  - GPSIMD cannot touch PSUM (BIR verifier): all evacuations are ACT/DVE,
    Pool gets SBUF-only work (masks, squares, onehot, small muls).
  - nested PSUM pools: prep pools close before the main loop so the
    histogram transpose pool gets 3 banks.
  - per-block engine steering; the last block's post-GEMM chain (the
    critical tail) leans on DVE 4x-mode masks + mostly-DVE evacuations.

Sharding: samples axis S split across 8 cores (1024 rows each); encodings/
categorical fully replicated per core; host concatenates the 8 outputs.

Engine budget per core at 158us: PE ~131us (GEMM 54.6 + fold 27.3 + mask
transposes 27.3 + prep), DVE ~131us (max8 candidates 76us is the immovable
item — InstMax has no 2x mode), ACT ~127us (PSUM evacuations), Pool ~69us.
"""

import os
import sys

import numpy as np

for _p in ("/opt/trn_rl_repo", "/root/.axon_site/_ro/trn_rl_repo"):
    if os.path.isdir(_p) and _p not in sys.path:
        sys.path.insert(0, _p)

import concourse.bass as bass
import concourse.mybir as mybir
from concourse import bacc, tile
from concourse.bass_utils import run_bass_kernel_spmd

F32 = mybir.dt.float32
F32R = mybir.dt.float32r
F16 = mybir.dt.float16
I32 = mybir.dt.int32

B, ENC, C, S, K = 8192, 256, 25, 8192, 25
EPS = 1e-5
NCORES = 8
SLOC = S // NCORES          # 1024 sample rows per core
NSB = SLOC // 128           # 8 sample blocks of 128 rows
NEB = B // 128              # 64 encoding blocks of 128 rows
NCH = B // 512              # 16 E-prep chunks of 512 columns
RECENTER = 176.0            # fp16 recenter: top-26 threshold lands near +31
NEG_BIG16 = -60000.0


def build_nc():
    nc = bacc.Bacc()
    enc_t = nc.declare_dram_parameter("enc", [B, ENC], F32R, isOutput=False)
    cat_t = nc.declare_dram_parameter("cat", [B, C], F32, isOutput=False)
    idx_t = nc.declare_dram_parameter("idx", [SLOC], I32, isOutput=False)
    ident_t = nc.declare_dram_parameter("ident", [128, 128], F32R, isOutput=False)
    out_t = nc.declare_dram_parameter("out", [SLOC], F32, isOutput=True)

    with tile.TileContext(nc) as tc:
        with (
            tc.tile_pool(name="persist", bufs=1) as persist,
            tc.tile_pool(name="ld", bufs=2) as ld,
            tc.tile_pool(name="sqp", bufs=2) as sqp,
            tc.tile_pool(name="small", bufs=2) as small,
            tc.tile_pool(name="xp", bufs=3) as xp,
            tc.tile_pool(name="mp", bufs=2) as mp,
            tc.tile_pool(name="mt", bufs=3) as mtp,
        ):
            # ---------------- persistent tiles ----------------
            et0s = [persist.tile([128, B // 4], F32R, tag=f"et0_{i}", name=f"et0_{i}")
                    for i in range(4)]
            et1s = [persist.tile([128, B // 4], F32R, tag=f"et1_{i}", name=f"et1_{i}")
                    for i in range(4)]
            onehot = persist.tile([128, NEB * C], F16, tag="onehot")
            qt0s = [persist.tile([128, 128], F32R, tag=f"qt0_{i}", name=f"qt0_{i}")
                    for i in range(NSB)]
            qt1s = [persist.tile([128, 128], F32R, tag=f"qt1_{i}", name=f"qt1_{i}")
                    for i in range(NSB)]
            e2row = persist.tile([1, B], F16, tag="e2row")  # 256 - e2[j]
            ones1 = persist.tile([1, 128], F16, tag="ones1")    # fold lhsT (+1)
            negcol = persist.tile([128, 1], F32R, tag="negcol")  # e2 reduce (-1)
            ident_sb = persist.tile([128, 128], F32R, tag="ident")
            ident_h = persist.tile([128, 128], F16, tag="identh")
            epsc = persist.tile([128, 1], F32, tag="epsc")
            negmg = persist.tile([128, NSB], F32, tag="negmg")
            outcol = persist.tile([128, NSB], F32, tag="outcol")
            idxall = persist.tile([128, NSB], I32, tag="idxall")
            qball = persist.tile([128, NSB, ENC], F32R, tag="qball")
            cqall = persist.tile([128, NSB, C], F32, tag="cqall")

            nc.scalar.dma_start(
                out=idxall[:],
                in_=idx_t[:].rearrange("(a p) -> p a", p=128),
            )
            nc.scalar.dma_start(out=ident_sb[:], in_=ident_t[:])
            nc.vector.tensor_copy(ident_h[:], ident_sb[:])
            nc.vector.memset(epsc[:], EPS)
            ones1f = persist.tile([1, 128], F32, tag="ones1f")
            nc.vector.memset(ones1f[:], 1.0)
            nc.scalar.activation(
                ones1[:], ones1f[:], mybir.ActivationFunctionType.Copy
            )
            negcf = persist.tile([128, 1], F32, tag="negcf")
            nc.vector.memset(negcf[:], -1.0)
            nc.scalar.activation(
                negcol[:], negcf[:], mybir.ActivationFunctionType.Copy
            )
            # preload the Ln activation table during startup slack so the
            # first entropy Ln doesn't stall ACT mid-stream
            lnwarm = persist.tile([128, 1], F32, tag="lnwarm")
            nc.scalar.activation(
                lnwarm[:], epsc[:], mybir.ActivationFunctionType.Ln,
                bias=epsc[:],
            )

            # xh quarter tiles per block (fp16 x+176); tags cycle with bufs=2.
            xhs = {}
            for s in range(NSB):
                xhs[s] = [xp.tile([128, B // 4], F16, tag=f"xh{i}",
                                  name=f"xh{s}_{i}") for i in range(4)]

            def emit_gemm_unit(s, u, eng_evac, pool):
                # GEMM for sample block s, columns 1024u..1024(u+1).
                pm = pool.tile([128, 1024], F32, tag="pmm")
                for half in range(2):
                    j0 = u * 1024 + half * 512
                    g4, to = j0 // 2048, j0 % 2048
                    po = half * 512
                    nc.tensor.matmul(
                        out=pm[:, po:po + 512], lhsT=qt0s[s][:],
                        rhs=et0s[g4][:, to:to + 512], start=True, stop=False,
                    )
                    nc.tensor.matmul(
                        out=pm[:, po:po + 512], lhsT=qt1s[s][:],
                        rhs=et1s[g4][:, to:to + 512], start=False, stop=False,
                    )
                    # K=1 fold of (256 - e2); the evac bias adds the
                    # remaining -80 of the fp16 recenter
                    nc.tensor.matmul(
                        out=pm[:, po:po + 512], lhsT=ones1[:],
                        rhs=e2row[:, j0:j0 + 512], start=False, stop=True,
                    )
                dst = xhs[s][u // 2][:, (u % 2) * 1024:(u % 2 + 1) * 1024]
                if eng_evac == "act":
                    nc.scalar.activation(
                        dst, pm[:], mybir.ActivationFunctionType.Copy,
                        bias=RECENTER - 256.0,
                    )
                else:
                    nc.vector.tensor_scalar(
                        out=dst, in0=pm[:], scalar1=RECENTER - 256.0,
                        scalar2=None, op0=mybir.AluOpType.add,
                    )

            def emit_q_prep(s):
                pq = ppt.tile([128, 512], F32R, tag="pt")
                for kc in range(2):
                    nc.tensor.transpose(
                        pq[:, kc * 128:(kc + 1) * 128],
                        qball[:, s, kc * 128:(kc + 1) * 128],
                        ident_sb[:],
                    )
                nc.scalar.activation(
                    qt0s[s][:], pq[:, 0:128],
                    mybir.ActivationFunctionType.Copy, scale=2.0,
                )
                nc.scalar.activation(
                    qt1s[s][:], pq[:, 128:256],
                    mybir.ActivationFunctionType.Copy, scale=2.0,
                )

            # ------------- prep: E chunks + q gathers, pipelined ------------
            with (
                tc.tile_pool(name="pt", bufs=3, space="PSUM") as ppt,
                tc.tile_pool(name="pe2", bufs=1, space="PSUM") as ppe2,
                tc.tile_pool(name="pmmP", bufs=2, space="PSUM") as pmmP,
            ):
                for t in range(NCH):
                    b0 = t * 4
                    eb = ld.tile([128, 4, ENC], F32R, tag="eb")
                    (nc.sync if t % 2 == 0 else nc.gpsimd).dma_start(
                        out=eb[:],
                        in_=enc_t[:]
                        .rearrange("(n p) k -> p n k", p=128)[:, b0:b0 + 4, :],
                    )
                    cb = ld.tile([128, 4, C], F32, tag="cb")
                    nc.sync.dma_start(
                        out=cb[:],
                        in_=cat_t[:]
                        .rearrange("(n p) k -> p n k", p=128)[:, b0:b0 + 4, :],
                    )
                    if t == 0:
                        # per-block row gathers (one [128,1]-offset indirect
                        # DMA per block)
                        for sb in range(NSB):
                            nc.gpsimd.indirect_dma_start(
                                out=qball[:, sb, :], out_offset=None,
                                in_=enc_t[:],
                                in_offset=bass.IndirectOffsetOnAxis(
                                    ap=idxall[:, sb:sb + 1], axis=0),
                            )
                    if t == 1:
                        emit_q_prep(0)
                        emit_q_prep(1)
                    g4, to = (t * 512) // 2048, (t * 512) % 2048
                    esqs = []
                    for kc, ets in ((0, et0s), (1, et1s)):
                        p = ppt.tile([128, 512], F32R, tag="pt")
                        for blk in range(4):
                            nc.tensor.transpose(
                                p[:, blk * 128:(blk + 1) * 128],
                                eb[:, blk, kc * 128:(kc + 1) * 128],
                                ident_sb[:],
                            )
                        dst = ets[g4][:, to:to + 512]
                        if t % 2 == 0:
                            nc.scalar.activation(
                                dst, p[:], mybir.ActivationFunctionType.Copy
                            )
                        else:
                            nc.vector.tensor_copy(dst, p[:])
                        esq = sqp.tile([128, 512], F32R, tag=f"esq{kc}")
                        nc.gpsimd.tensor_tensor(
                            out=esq[:], in0=dst, in1=dst,
                            op=mybir.AluOpType.mult,
                        )
                        esqs.append(esq)
                    nc.gpsimd.tensor_tensor(
                        out=esqs[0][:], in0=esqs[0][:], in1=esqs[1][:],
                        op=mybir.AluOpType.add,
                    )
                    # -e2 for this chunk: (-1)-column reduces partitions
                    pe2 = ppe2.tile([1, 512], F32, tag="pe2")
                    nc.tensor.matmul(
                        out=pe2[:], lhsT=negcol[:], rhs=esqs[0][:],
                        start=True, stop=True,
                    )
                    nc.scalar.activation(
                        e2row[:, t * 512:(t + 1) * 512], pe2[:],
                        mybir.ActivationFunctionType.Copy, bias=256.0,
                    )
                    # onehot[j, c] = (cat[j, c] == rowmax) — value-match
                    # (ties are measure-zero in the uniform categorical input)
                    mxv = small.tile([128, 4], F32, tag="mxv")
                    nc.vector.reduce_max(mxv[:], cb[:], axis=mybir.AxisListType.X)
                    for blk in range(4):
                        b = b0 + blk
                        nc.gpsimd.tensor_scalar(
                            out=onehot[:, b * C:(b + 1) * C],
                            in0=cb[:, blk, :],
                            scalar1=mxv[:, blk:blk + 1],
                            scalar2=None,
                            op0=mybir.AluOpType.is_equal,
                        )
                    if 2 <= t <= 7:
                        emit_q_prep(t)
                    if t == 8:
                        for sb in range(NSB):
                            nc.gpsimd.indirect_dma_start(
                                out=cqall[:, sb, :], out_offset=None,
                                in_=cat_t[:],
                                in_offset=bass.IndirectOffsetOnAxis(
                                    ap=idxall[:, sb:sb + 1], axis=0),
                            )
                    # blocks 0/1 GEMM for the finished chunk pair
                    if t % 2 == 1:
                        u = t // 2
                        emit_gemm_unit(0, u, "dve" if u % 4 == 1 else "act", pmmP)
                        emit_gemm_unit(1, u, "dve" if u % 4 == 3 else "act", pmmP)

                # per-block confidence -max(categorical[idxs])
                for s in range(NSB):
                    mg = small.tile([128, 1], F32, tag="mg")
                    nc.vector.reduce_max(
                        mg[:], cqall[:, s, :], axis=mybir.AxisListType.X
                    )
                    nc.gpsimd.tensor_scalar(
                        out=negmg[:, s:s + 1], in0=mg[:],
                        scalar1=-1.0, scalar2=None, op0=mybir.AluOpType.mult,
                    )

            # ---------------- main: per sample block ----------------
            with (
                tc.tile_pool(name="pmmM", bufs=2, space="PSUM") as pmmM,
                tc.tile_pool(name="ptb", bufs=3, space="PSUM") as pptb,
                tc.tile_pool(name="pcnt", bufs=1, space="PSUM") as pcnt,
            ):
                for s in range(NSB):
                    last = s == NSB - 1
                    if s >= 2:
                        for u in range(8):
                            emit_gemm_unit(s, u, "act", pmmM)
                    xh = xhs[s]

                    # exact top-26 (largest x == nearest) per row, on fp16
                    cand = small.tile([128, 128], F16, tag="cand")
                    for c in range(16):
                        nc.vector.max(
                            out=cand[:, c * 8:(c + 1) * 8],
                            in_=xh[c // 4][:, (c % 4) * 512:(c % 4 + 1) * 512],
                        )
                    top32 = small.tile([128, 32], F16, tag="top32")
                    for r in range(4):
                        nc.vector.max(out=top32[:, r * 8:(r + 1) * 8], in_=cand[:])
                        if r < 3:
                            nc.vector.match_replace(
                                out=cand[:],
                                in_to_replace=top32[:, r * 8:(r + 1) * 8],
                                in_values=cand[:],
                                imm_value=NEG_BIG16,
                            )
                    t32 = small.tile([128, 1], F32, tag="t32")
                    nc.vector.tensor_copy(t32[:], top32[:, 25:26])

                    # strict mask vs the 26th-largest value, exact fp16 0/1.
                    # Steady state: Pool (keeps DVE for max8).  Last block:
                    # the mask is on the critical tail — use DVE's 4x mode.
                    mask = [mp.tile([128, B // 4], F16, tag=f"mk{i}",
                                    name=f"mk{s}_{i}") for i in range(4)]
                    for qi in range(4):
                        if s < 2:
                            eng = nc.vector if qi % 2 == 0 else nc.gpsimd
                        elif last:
                            eng = nc.vector if qi < 3 else nc.gpsimd
                        else:
                            eng = nc.gpsimd
                        eng.tensor_scalar(
                            out=mask[qi][:],
                            in0=xh[qi][:],
                            scalar1=t32[:],
                            scalar2=None,
                            op0=mybir.AluOpType.is_gt,
                        )

                    # counts[r, c] = sum_j mask[r, j] * onehot[j, c]
                    pc = pcnt.tile([128, C], F32, tag="pcnt")
                    for g in range(8):
                        ptm = pptb.tile([128, 1024], F16, tag="ptb")
                        qi, off = g // 2, (g % 2) * 1024
                        for i in range(8):
                            nc.tensor.transpose(
                                ptm[:, i * 128:(i + 1) * 128],
                                mask[qi][:, off + i * 128:off + (i + 1) * 128],
                                ident_h[:],
                            )
                        mtt = mtp.tile([128, 1024], F16, tag="mt")
                        on_act = (g % 8 in (0, 3, 6)) if last else (g % 2 == 0)
                        if on_act:
                            nc.scalar.activation(
                                mtt[:], ptm[:], mybir.ActivationFunctionType.Copy
                            )
                        else:
                            nc.vector.tensor_copy(mtt[:], ptm[:])
                        for i in range(8):
                            b = g * 8 + i
                            nc.tensor.matmul(
                                out=pc[:],
                                lhsT=mtt[:, i * 128:(i + 1) * 128],
                                rhs=onehot[:, b * C:(b + 1) * C],
                                start=(b == 0), stop=(b == NEB - 1),
                            )

                    counts = small.tile([128, C], F32, tag="counts")
                    nsum = small.tile([128, 1], F32, tag="nsum")
                    nc.scalar.activation(
                        counts[:], pc[:], mybir.ActivationFunctionType.Copy,
                        accum_out=nsum[:],
                    )
                    rn = small.tile([128, 1], F32, tag="rn")
                    nc.vector.reciprocal(rn[:], nsum[:])
                    p_t = small.tile([128, C], F32, tag="p")
                    nc.gpsimd.tensor_scalar(
                        out=p_t[:], in0=counts[:],
                        scalar1=rn[:], scalar2=None, op0=mybir.AluOpType.mult,
                    )
                    lg = small.tile([128, C], F32, tag="lg")
                    nc.scalar.activation(
                        lg[:], p_t[:], mybir.ActivationFunctionType.Ln,
                        bias=epsc[:],
                    )
                    pl = small.tile([128, C], F32, tag="pl")
                    nc.gpsimd.tensor_tensor(
                        out=pl[:], in0=p_t[:], in1=lg[:],
                        op=mybir.AluOpType.mult,
                    )
                    ent = small.tile([128, 1], F32, tag="ent")
                    nc.vector.reduce_sum(ent[:], pl[:], axis=mybir.AxisListType.X)
                    nc.gpsimd.tensor_tensor(
                        out=outcol[:, s:s + 1],
                        in0=ent[:],
                        in1=negmg[:, s:s + 1],
                        op=mybir.AluOpType.mult,
                    )

                nc.sync.dma_start(
                    out=out_t[:].rearrange("(b p) -> p b", p=128),
                    in_=outcol[:],
                )

    nc.finalize()
    return nc


_NC_CACHE = {}


def _get_nc():
    if "nc" not in _NC_CACHE:
        _NC_CACHE["nc"] = build_nc()
    return _NC_CACHE["nc"]


def _make_in_maps(encodings, categorical, idxs):
    enc = np.ascontiguousarray(np.asarray(encodings, dtype=np.float32))
    cat = np.ascontiguousarray(np.asarray(categorical, dtype=np.float32))
    idx = np.ascontiguousarray(np.asarray(idxs, dtype=np.int32))
    ident = np.eye(128, dtype=np.float32)
    in_maps = []
    for c in range(NCORES):
        in_maps.append({
            "enc": enc,
            "cat": cat,
            "idx": idx[c * SLOC:(c + 1) * SLOC],
            "ident": ident,
        })
    return in_maps


def run(encodings, categorical, idxs, trace=False):
    """Run the SPMD kernel; returns (out [S] f32, BassKernelResults)."""
    nc = _get_nc()
    in_maps = _make_in_maps(encodings, categorical, idxs)
    res = run_bass_kernel_spmd(
        nc, in_maps, core_ids=list(range(NCORES)), trace=trace
    )
    out = np.concatenate(
        [np.asarray(res.results[c]["out"], dtype=np.float32)
         for c in range(NCORES)]
    )
    return out, res


def kernel(encodings, categorical, idxs):
    out, _ = run(encodings, categorical, idxs)
    return out
